# revision 14
# baseline (speedup 1.0000x reference)
"""DenseGeneralAqt inference kernel for Trainium2 (8 NeuronCores).

out = (x @ dequant_int8(qkernel)) * qscale,  x:(2,2048,1024) f32,
qkernel:(1024,4096) int8, qscale:(1,4096) f32 -> out:(2,2048,4096) f32.

Strategy: 2D sharding — 4-way over the flattened token axis (M) x 2-way
over features (N); per core M=1024, K=1024, N=2048. Host marshalling
transposes x to [D, M] fp16 (contraction on SBUF partitions) in the
same pass that shards it.

The kernel is paced by two hard limits: the PE matmul stream (256
matmuls x ~215 ns at 2.4 GHz = 55 us) and early-HBM delivery
(~130-150 GB/s per HWDGE ring during the 8-core startup burst). The
compute covers the M x N block in four (m-quad x n-half) sweeps of 8
PSUM banks, k-outer, so sweep 0 consumes only 128 KB of weights + 128
KB of activations per 1.72 us round — matched to what the rings
actually deliver. Weights ride the Scalar HWDGE ring as half-tiles in
(half, k) order; activations ride the Sync ring sliced by m-quad with
k-tile 0 leading. Weight k-tiles dequantize int8 -> fp16 on the vector
engine just ahead of PE consumption (k-tile 0's first half in quarters
— the first matmul is gated by a 512-column cast). A PE warm-up on
zeros bridges the DMA-landing window so the HAM clock-gate (needs ~3.4
us of sustained PE busy) releases right when real matmuls start; any
mid-kernel PE gap > ~3.4 us re-throttles the clock to 1.2 GHz, so the
schedule keeps all stalls well under that. (The GpSimd SWDGE ring is
used only for the deferred per-channel-scale broadcast: measured ~12
us to first-land and it starves the HWDGE rings when given real work.)

Drains fuse the per-channel scale into the PSUM->SBUF pass on the
vector engine and store fp16 (upcast to f32 on host; adds ~2e-4
relative error against a 2e-2 budget, halves output HBM traffic). The
last sweep runs n-outer so its drains overlap the remaining matmuls,
stores alternate between the Scalar and Sync rings, and the final two
banks drain in column chunks so the last HBM write receipt is small.
"""

import numpy as np

P = 128
B, S, D, F = 2, 2048, 1024, 4096
N_CORES = 8
MSH, NSH = 4, 2                   # shard grid: 4 m-blocks x 2 n-blocks
M_FULL = B * S                    # 4096 rows
M_CORE = M_FULL // MSH            # 1024 rows per core
N_CORE = F // NSH                 # 2048 cols per core
NT = 512                          # n-tile (one PSUM bank of f32)
WM, WK, WN = M_CORE // P, D // P, N_CORE // NT

_CACHE: dict = {}


def _build():
    import concourse.tile as tile
    from concourse import bacc, mybir

    nc = bacc.Bacc("TRN2", target_bir_lowering=False, debug=False)

    xt_dram = nc.dram_tensor("xt", [D, M_CORE], mybir.dt.float16, kind="ExternalInput")
    w_dram = nc.dram_tensor("w", [D, N_CORE], mybir.dt.int8, kind="ExternalInput")
    s_dram = nc.dram_tensor("s", [1, N_CORE], mybir.dt.float32, kind="ExternalInput")
    o_dram = nc.dram_tensor("o", [M_CORE, N_CORE], mybir.dt.float16, kind="ExternalOutput")

    xt_view = xt_dram[:, :].rearrange("(kt kp) m -> kp kt m", kp=P)  # [128, 8, 1024]

    with tile.TileContext(nc) as tc:
        with (
            tc.tile_pool(name="wi", bufs=1) as wip,
            tc.tile_pool(name="w", bufs=1) as wp,
            tc.tile_pool(name="qs", bufs=1) as qp,
            tc.tile_pool(name="xh", bufs=1) as xhp,
            tc.tile_pool(name="o", bufs=12) as op,
            tc.tile_pool(name="ps", bufs=8, space="PSUM") as pp,
        ):
            w_i8 = [
                wip.tile([P, N_CORE], mybir.dt.int8, name=f"wi{kt}", tag=f"wi{kt}")
                for kt in range(WK)
            ]
            wh = N_CORE // 2
            xh = xhp.tile([P, WK, M_CORE], mybir.dt.float16, name="xh", tag="xh")
            MQ = 4 * P  # m-columns per quad-sweep

            # Scalar ring: weight half-tiles, first halves (n columns
            # 0..1023, sweeps 0/2) in k order, then second halves. Sweep 0
            # consumes one 128 KB half per 1.72 us round; the second halves
            # aren't touched until sweep 1 (~25 us), so their DMAs can lag.
            for half in range(2):
                sl = slice(half * wh, (half + 1) * wh)
                for kt in range(WK):
                    nc.scalar.dma_start(w_i8[kt][:, sl], w_dram[kt * P:(kt + 1) * P, sl])

            # Sync ring: x sliced by m-quad (sweeps 0/1 touch only the first
            # 512 m-columns); quad A's leading k-tiles go as small DMAs.
            nc.sync.dma_start(xh[:, 0:1, 0:MQ], xt_view[:, 0:1, 0:MQ])
            nc.sync.dma_start(xh[:, 1:2, 0:MQ], xt_view[:, 1:2, 0:MQ])
            nc.sync.dma_start(xh[:, 2:WK, 0:MQ], xt_view[:, 2:WK, 0:MQ])
            nc.sync.dma_start(xh[:, :, MQ:M_CORE], xt_view[:, :, MQ:M_CORE])

            # PE warm-up on zeros while the first loads are in flight: long
            # enough to bridge to the first real matmul (~11 us) so the HAM
            # clock-gate releases right as the real stream begins. The
            # memset rides GpSimd so the warm-up has no vector dependency.
            warm = wp.tile([P, NT], mybir.dt.float16, name="warm", tag="warm")
            nc.gpsimd.memset(warm[:], 0)
            warm_ps = pp.tile([P, NT], mybir.dt.float32, name="warm_ps", tag="ps")
            for _ in range(32):
                nc.tensor.matmul(warm_ps[:, 0:P], warm[:, 0:P], warm[:, 0:P])

            # Dequant int8 -> fp16 on the vector engine in DMA-landing
            # order; k-tile 0's first half in quarters so the very first
            # matmul is gated by a 512-column cast. CRITICAL: the vector
            # engine is strict-FIFO, so only casts whose DMAs land before
            # sweep 0 ends may be emitted ahead of sweep 0's drains — a
            # cast stuck on an unlanded DMA would block the drains behind
            # it, stall PSUM recycling, and re-throttle the PE clock. The
            # second-half casts for k>=2 are emitted after sweep 0 instead.
            w_sb = [
                wp.tile([P, N_CORE], mybir.dt.float16, name=f"w{kt}", tag=f"w{kt}")
                for kt in range(WK)
            ]
            wq = N_CORE // 4

            def cast_half(kt, half):
                sl = slice(half * wh, (half + 1) * wh)
                return nc.vector.tensor_copy(w_sb[kt][:, sl], w_i8[kt][:, sl])

            nc.vector.tensor_copy(w_sb[0][:, 0:wq], w_i8[0][:, 0:wq])
            nc.vector.tensor_copy(w_sb[0][:, wq:wh], w_i8[0][:, wq:wh])
            cv = [cast_half(kt, 0) for kt in range(1, WK)]
            cast_half(0, 1)
            cast_half(1, 1)

            # Scale broadcast (1 MB DRE replication) on the GpSimd SWDGE
            # ring; deferred so its bytes don't starve the critical early
            # loads. Needed only at the first drain (~25 us in).
            qs = qp.tile([P, N_CORE], mybir.dt.float32)
            qs_dma = nc.gpsimd.dma_start(qs[:], s_dram[0:1, :].to_broadcast((P, N_CORE)))
            tile.add_dep_helper(qs_dma.ins, cv[1].ins, reason="defer qs broadcast")

            def drain(mi, nt, ps_tile, eng, chunks=1):
                # PSUM -> (xqscale) -> SBUF fp16 -> DRAM. chunks>1 splits the
                # drain column-wise so the final HBM write receipt is small.
                cw = NT // chunks
                for c in range(chunks):
                    ot = op.tile([P, cw], mybir.dt.float16, name=f"o{mi}_{nt}_{c}", tag="o")
                    sl = slice(nt * NT + c * cw, nt * NT + (c + 1) * cw)
                    nc.vector.tensor_mul(ot[:], ps_tile[:, c * cw:(c + 1) * cw], qs[:, sl])
                    e = eng if chunks == 1 else (nc.sync if c % 2 == 0 else nc.scalar)
                    e.dma_start(o_dram[mi * P:(mi + 1) * P, sl], ot[:])

            def mm(ps_tile, kt, mi, nt, first, last):
                nc.tensor.matmul(
                    ps_tile[:],
                    xh[:, kt, mi * P:(mi + 1) * P],
                    w_sb[kt][:, nt * NT:(nt + 1) * NT],
                    start=first,
                    stop=last,
                )

            # Four (m-quad x n-half) sweeps of 8 PSUM banks, k-outer; the
            # n-half progression means each sweep's weights are a 1 MB
            # half-stream that landed during the previous sweep.
            sweeps = [
                (range(0, 4), (0, 1)),
                (range(0, 4), (2, 3)),
                (range(4, 8), (0, 1)),
                (range(4, 8), (2, 3)),
            ]
            for si, (quad, nh) in enumerate(sweeps):
                combos = [(mi, nt) for mi in quad for nt in nh]
                if si < len(sweeps) - 1:
                    ps = {
                        c: pp.tile([P, NT], mybir.dt.float32, name=f"ps{si}_{c[0]}_{c[1]}", tag="ps")
                        for c in combos
                    }
                    for kt in range(WK):
                        # n-minor: each n-block's matmuls gate only on the
                        # piecewise dequant of its own columns.
                        for n in nh:
                            for m in quad:
                                mm(ps[(m, n)], kt, m, n, kt == 0, kt == WK - 1)
                    for ci, c in enumerate(combos):
                        eng = nc.scalar if ci % 2 == 0 else nc.sync
                        drain(c[0], c[1], ps[c], eng)
                    if si == 0:
                        # Second-half casts for k>=2 (their DMAs land while
                        # sweep 0 computes; emitted here so they queue on
                        # the vector engine behind sweep 0's drains).
                        for kt in range(2, WK):
                            cast_half(kt, 1)
                else:
                    # Last sweep: n-outer so each bank's reduction finishes
                    # early and its drain overlaps the remaining matmuls;
                    # stores alternate rings, and the last two banks drain
                    # in column chunks to shrink the completion tail.
                    for ci, c in enumerate(combos):
                        ps_t = pp.tile([P, NT], mybir.dt.float32, name=f"ps{si}_{c[0]}_{c[1]}", tag="ps")
                        for kt in range(WK):
                            mm(ps_t, kt, c[0], c[1], kt == 0, kt == WK - 1)
                        chunks = 1 if ci < 6 else (2 if ci == 6 else 4)
                        eng = nc.scalar if ci % 2 == 0 else nc.sync
                        drain(c[0], c[1], ps_t, eng, chunks=chunks)

    nc.compile()
    return nc


def _get_nc():
    if "nc" not in _CACHE:
        _CACHE["nc"] = _build()
    return _CACHE["nc"]


def _run(x, qkernel, qscale, trace=False):
    from concourse.bass_utils import run_bass_kernel_spmd

    x = np.asarray(x, dtype=np.float32).reshape(M_FULL, D)
    xt = np.ascontiguousarray(x.T).astype(np.float16)    # [D, M_FULL]
    w = np.asarray(qkernel)
    if w.dtype != np.int8:
        w = w.astype(np.int8)
    s = np.asarray(qscale, dtype=np.float32).reshape(1, F)

    in_maps = []
    for c in range(N_CORES):
        mb, nb = c % MSH, c // MSH
        in_maps.append({
            "xt": np.ascontiguousarray(xt[:, mb * M_CORE:(mb + 1) * M_CORE]),
            "w": np.ascontiguousarray(w[:, nb * N_CORE:(nb + 1) * N_CORE]),
            "s": np.ascontiguousarray(s[:, nb * N_CORE:(nb + 1) * N_CORE]),
        })
    res = run_bass_kernel_spmd(
        _get_nc(), in_maps, core_ids=list(range(N_CORES)), trace=trace
    )
    out = np.empty((M_FULL, F), dtype=np.float32)
    for c in range(N_CORES):
        mb, nb = c % MSH, c // MSH
        out[mb * M_CORE:(mb + 1) * M_CORE, nb * N_CORE:(nb + 1) * N_CORE] = (
            res.results[c]["o"].astype(np.float32)
        )
    return out.reshape(B, S, F), res


def kernel(x, qkernel, qscale):
    try:
        out, _ = _run(x, qkernel, qscale, trace=False)
    except Exception:
        # One retry for transient device-side failures.
        out, _ = _run(x, qkernel, qscale, trace=False)
    return out


def kernel_traced(x, qkernel, qscale):
    out, res = _run(x, qkernel, qscale, trace=True)
    return out, res


# revision 17
# speedup vs baseline: 1.1088x; 1.1088x over previous
"""DenseGeneralAqt inference kernel for Trainium2 (8 NeuronCores).

out = (x @ dequant_int8(qkernel)) * qscale,  x:(2,2048,1024) f32,
qkernel:(1024,4096) int8, qscale:(1,4096) f32 -> out:(2,2048,4096) f32.

Strategy: 2D sharding — 4-way over the flattened token axis (M) x 2-way
over features (N); per core M=1024, K=1024, N=2048. Host marshalling
transposes x to [D, M] fp16 (contraction on SBUF partitions) in the
same pass that shards it.

The kernel is paced by two hard limits: the PE matmul stream (256
matmuls x ~215 ns at 2.4 GHz = 55 us) and early-HBM delivery
(~130-150 GB/s per HWDGE ring during the 8-core startup burst). The
compute covers the M x N block in four (m-quad x n-half) sweeps of 8
PSUM banks, k-outer, so sweep 0 consumes only 128 KB of weights + 128
KB of activations per 1.72 us round — matched to what the rings
actually deliver. Weights ride the Scalar HWDGE ring as half-tiles in
(half, k) order; activations ride the Sync ring sliced by m-quad with
k-tile 0 leading. Weight k-tiles dequantize int8 -> fp16 on the vector
engine just ahead of PE consumption (k-tile 0's first half in quarters
— the first matmul is gated by a 512-column cast). A PE warm-up on
zeros bridges the DMA-landing window so the HAM clock-gate (needs ~3.4
us of sustained PE busy) releases right when real matmuls start; any
mid-kernel PE gap > ~3.4 us re-throttles the clock to 1.2 GHz, so the
schedule keeps all stalls well under that. (The GpSimd SWDGE ring is
used only for the deferred per-channel-scale broadcast: measured ~12
us to first-land and it starves the HWDGE rings when given real work.)

Drains fuse the per-channel scale into the PSUM->SBUF pass on the
vector engine and store fp16 (upcast to f32 on host; adds ~2e-4
relative error against a 2e-2 budget, halves output HBM traffic). The
last sweep runs n-outer so its drains overlap the remaining matmuls,
stores alternate between the Scalar and Sync rings, and the final two
banks drain in column chunks so the last HBM write receipt is small.
"""

import numpy as np

P = 128
B, S, D, F = 2, 2048, 1024, 4096
N_CORES = 8
MSH, NSH = 4, 2                   # shard grid: 4 m-blocks x 2 n-blocks
M_FULL = B * S                    # 4096 rows
M_CORE = M_FULL // MSH            # 1024 rows per core
N_CORE = F // NSH                 # 2048 cols per core
NT = 512                          # n-tile (one PSUM bank of f32)
WM, WK, WN = M_CORE // P, D // P, N_CORE // NT

_CACHE: dict = {}


def _build():
    import concourse.tile as tile
    from concourse import bacc, mybir

    nc = bacc.Bacc("TRN2", target_bir_lowering=False, debug=False)

    xt_dram = nc.dram_tensor("xt", [D, M_CORE], mybir.dt.float16, kind="ExternalInput")
    w_dram = nc.dram_tensor("w", [D, N_CORE], mybir.dt.int8, kind="ExternalInput")
    s_dram = nc.dram_tensor("s", [1, N_CORE], mybir.dt.float32, kind="ExternalInput")
    o_dram = nc.dram_tensor("o", [M_CORE, N_CORE], mybir.dt.float16, kind="ExternalOutput")

    xt_view = xt_dram[:, :].rearrange("(kt kp) m -> kp kt m", kp=P)  # [128, 8, 1024]

    with tile.TileContext(nc) as tc:
        with (
            tc.tile_pool(name="wi", bufs=1) as wip,
            tc.tile_pool(name="w", bufs=1) as wp,
            tc.tile_pool(name="qs", bufs=1) as qp,
            tc.tile_pool(name="xh", bufs=1) as xhp,
            tc.tile_pool(name="o", bufs=12) as op,
            tc.tile_pool(name="ps", bufs=8, space="PSUM") as pp,
        ):
            w_i8 = [
                wip.tile([P, N_CORE], mybir.dt.int8, name=f"wi{kt}", tag=f"wi{kt}")
                for kt in range(WK)
            ]
            wh = N_CORE // 2
            xh = xhp.tile([P, WK, M_CORE], mybir.dt.float16, name="xh", tag="xh")
            MQ = 4 * P  # m-columns per quad-sweep

            # Scalar ring: weight half-tiles, first halves (n columns
            # 0..1023, sweeps 0/2) in k order, then second halves. Sweep 0
            # consumes one 128 KB half per 1.72 us round; the second halves
            # aren't touched until sweep 1 (~25 us), so their DMAs can lag.
            for half in range(2):
                sl = slice(half * wh, (half + 1) * wh)
                for kt in range(WK):
                    nc.scalar.dma_start(w_i8[kt][:, sl], w_dram[kt * P:(kt + 1) * P, sl])

            # Sync ring: x sliced by m-quad (sweeps 0/1 touch only the first
            # 512 m-columns); quad A's leading k-tiles go as small DMAs.
            nc.sync.dma_start(xh[:, 0:1, 0:MQ], xt_view[:, 0:1, 0:MQ])
            nc.sync.dma_start(xh[:, 1:2, 0:MQ], xt_view[:, 1:2, 0:MQ])
            nc.sync.dma_start(xh[:, 2:WK, 0:MQ], xt_view[:, 2:WK, 0:MQ])
            nc.sync.dma_start(xh[:, :, MQ:M_CORE], xt_view[:, :, MQ:M_CORE])

            # PE warm-up on zeros while the first loads are in flight: long
            # enough to bridge to the first real matmul (~11 us) so the HAM
            # clock-gate releases right as the real stream begins. The
            # memset rides GpSimd so the warm-up has no vector dependency.
            warm = wp.tile([P, NT], mybir.dt.float16, name="warm", tag="warm")
            nc.gpsimd.memset(warm[:], 0)
            warm_ps = pp.tile([P, NT], mybir.dt.float32, name="warm_ps", tag="ps")
            for _ in range(32):
                nc.tensor.matmul(warm_ps[:, 0:P], warm[:, 0:P], warm[:, 0:P])

            # Dequant int8 -> fp16 on the vector engine in DMA-landing
            # order; k-tile 0's first half in quarters so the very first
            # matmul is gated by a 512-column cast. CRITICAL: the vector
            # engine is strict-FIFO, so only casts whose DMAs land before
            # sweep 0 ends may be emitted ahead of sweep 0's drains — a
            # cast stuck on an unlanded DMA would block the drains behind
            # it, stall PSUM recycling, and re-throttle the PE clock. The
            # second-half casts for k>=2 are emitted after sweep 0 instead.
            w_sb = [
                wp.tile([P, N_CORE], mybir.dt.float16, name=f"w{kt}", tag=f"w{kt}")
                for kt in range(WK)
            ]
            wq = N_CORE // 4

            def cast_half(kt, half):
                sl = slice(half * wh, (half + 1) * wh)
                return nc.vector.tensor_copy(w_sb[kt][:, sl], w_i8[kt][:, sl])

            nc.vector.tensor_copy(w_sb[0][:, 0:wq], w_i8[0][:, 0:wq])
            nc.vector.tensor_copy(w_sb[0][:, wq:wh], w_i8[0][:, wq:wh])
            cv = [cast_half(kt, 0) for kt in range(1, WK)]

            # Scale broadcast (1 MB DRE replication) on the GpSimd SWDGE
            # ring; deferred so its bytes don't starve the critical early
            # loads. Needed only at the first drain (~25 us in).
            qs = qp.tile([P, N_CORE], mybir.dt.float32)
            qs_dma = nc.gpsimd.dma_start(qs[:], s_dram[0:1, :].to_broadcast((P, N_CORE)))
            tile.add_dep_helper(qs_dma.ins, cv[1].ins, reason="defer qs broadcast")

            def drain(mi, nt, ps_tile, eng, chunks=1):
                # PSUM -> (xqscale) -> SBUF fp16 -> DRAM. chunks>1 splits the
                # drain column-wise so the final HBM write receipt is small.
                cw = NT // chunks
                muls = []
                for c in range(chunks):
                    ot = op.tile([P, cw], mybir.dt.float16, name=f"o{mi}_{nt}_{c}", tag="o")
                    sl = slice(nt * NT + c * cw, nt * NT + (c + 1) * cw)
                    muls.append(
                        nc.vector.tensor_mul(ot[:], ps_tile[:, c * cw:(c + 1) * cw], qs[:, sl])
                    )
                    e = eng if chunks == 1 else (nc.sync if c % 2 == 0 else nc.scalar)
                    e.dma_start(o_dram[mi * P:(mi + 1) * P, sl], ot[:])
                return muls

            def mm(ps_tile, kt, mi, nt, first, last):
                nc.tensor.matmul(
                    ps_tile[:],
                    xh[:, kt, mi * P:(mi + 1) * P],
                    w_sb[kt][:, nt * NT:(nt + 1) * NT],
                    start=first,
                    stop=last,
                )

            # Four (m-quad x n-half) sweeps of 8 PSUM banks, k-outer. BOTH
            # m-quads consume the first weight halves before either touches
            # the second halves: the sweep 0 -> 1 boundary then needs no new
            # dequants (h0 is reused from SBUF), and the h1 casts aren't
            # needed until sweep 2 (~40 us) — far after their DMAs land.
            sweeps = [
                (range(0, 4), (0, 1)),
                (range(4, 8), (0, 1)),
                (range(0, 4), (2, 3)),
                (range(4, 8), (2, 3)),
            ]
            for si, (quad, nh) in enumerate(sweeps):
                combos = [(mi, nt) for mi in quad for nt in nh]
                if si < len(sweeps) - 1:
                    ps = {
                        c: pp.tile([P, NT], mybir.dt.float32, name=f"ps{si}_{c[0]}_{c[1]}", tag="ps")
                        for c in combos
                    }
                    for kt in range(WK):
                        # n-minor: each n-block's matmuls gate only on the
                        # piecewise dequant of its own columns.
                        for n in nh:
                            for m in quad:
                                mm(ps[(m, n)], kt, m, n, kt == 0, kt == WK - 1)
                    last_mul = None
                    for ci, c in enumerate(combos):
                        eng = nc.scalar if ci % 2 == 0 else nc.sync
                        last_mul = drain(c[0], c[1], ps[c], eng)[-1]
                    if si == 0:
                        # Second-half casts: their DMAs land ~25-30 us, and
                        # sweep 2 needs them from ~40 us. Pin them BEHIND
                        # sweep 0's drains — the vector engine is strict
                        # FIFO, and a hoisted cast stuck on an unlanded DMA
                        # would block the drains, stall PSUM recycling, and
                        # re-throttle the PE clock (HAM).
                        for kt in range(WK):
                            cva = cast_half(kt, 1)
                            tile.add_dep_helper(
                                cva.ins, last_mul.ins, reason="casts after drains"
                            )
                else:
                    # Last sweep: n-outer so each bank's reduction finishes
                    # early and its drain overlaps the remaining matmuls;
                    # stores alternate rings, and the last two banks drain
                    # in column chunks to shrink the completion tail.
                    for ci, c in enumerate(combos):
                        ps_t = pp.tile([P, NT], mybir.dt.float32, name=f"ps{si}_{c[0]}_{c[1]}", tag="ps")
                        for kt in range(WK):
                            mm(ps_t, kt, c[0], c[1], kt == 0, kt == WK - 1)
                        chunks = 1 if ci < 6 else (2 if ci == 6 else 4)
                        eng = nc.scalar if ci % 2 == 0 else nc.sync
                        drain(c[0], c[1], ps_t, eng, chunks=chunks)

    nc.compile()
    return nc


def _get_nc():
    if "nc" not in _CACHE:
        _CACHE["nc"] = _build()
    return _CACHE["nc"]


def _run(x, qkernel, qscale, trace=False):
    from concourse.bass_utils import run_bass_kernel_spmd

    x = np.asarray(x, dtype=np.float32).reshape(M_FULL, D)
    xt = np.ascontiguousarray(x.T).astype(np.float16)    # [D, M_FULL]
    w = np.asarray(qkernel)
    if w.dtype != np.int8:
        w = w.astype(np.int8)
    s = np.asarray(qscale, dtype=np.float32).reshape(1, F)

    in_maps = []
    for c in range(N_CORES):
        mb, nb = c % MSH, c // MSH
        in_maps.append({
            "xt": np.ascontiguousarray(xt[:, mb * M_CORE:(mb + 1) * M_CORE]),
            "w": np.ascontiguousarray(w[:, nb * N_CORE:(nb + 1) * N_CORE]),
            "s": np.ascontiguousarray(s[:, nb * N_CORE:(nb + 1) * N_CORE]),
        })
    res = run_bass_kernel_spmd(
        _get_nc(), in_maps, core_ids=list(range(N_CORES)), trace=trace
    )
    out = np.empty((M_FULL, F), dtype=np.float32)
    for c in range(N_CORES):
        mb, nb = c % MSH, c // MSH
        out[mb * M_CORE:(mb + 1) * M_CORE, nb * N_CORE:(nb + 1) * N_CORE] = (
            res.results[c]["o"].astype(np.float32)
        )
    return out.reshape(B, S, F), res


def kernel(x, qkernel, qscale):
    try:
        out, _ = _run(x, qkernel, qscale, trace=False)
    except Exception:
        # One retry for transient device-side failures.
        out, _ = _run(x, qkernel, qscale, trace=False)
    return out


def kernel_traced(x, qkernel, qscale):
    out, res = _run(x, qkernel, qscale, trace=True)
    return out, res
